# revision 1
# baseline (speedup 1.0000x reference)
"""CXLoss kernel for trn2 (8 NeuronCores).

Math
----
reference computes, per sample n:
  dot[q,p]   = <Tn[:,q], In[:,p]>          (C=256 contraction, P=4096)
  raw        = (1-dot)/2
  mn[p]      = min_q raw[q,p];  denom = mn + eps
  w[q,p]     = exp((1 - raw/denom)/sigma)
  cx_feat    = w / sum_q w
  cx         = 0.5*cx_feat + 0.5*cx_sp
  loss       = mean_n -log(mean_q max_p cx[q,p])

The spatial branch cx_sp is input-independent and, in fp32, is EXACTLY the
identity matrix: the spatial distance matrix has 0 on the diagonal, so
mn=0, denom=eps=1e-5 and the softmax temperature collapses — every
off-diagonal weight underflows to 0.  Hence
  max_p cx[q,p] = 0.5 + 0.5*cx_feat[q,q]
and only the DIAGONAL of cx_feat (plus the column sums S) is needed:
  loss = mean_n(-log(mean_q(0.5 + 0.5*w[q,q]/S[q])))
(verified vs reference: rel err ~4e-7).

Fold raw/mn into dot:  w = exp(dot*s_p + b_p) with
  denom = (1-mx_p)/2 + eps   (mx_p = max_q dot[q,p])
  s_p = 5/denom,  b_p = 10 - s_p       (sigma=0.1, B=1)

Sharding: 8 cores = 2 samples x 4 slices of the p axis (1024 rows each).
Per core, 8 p-blocks of 128 rows x 4096 q columns. q is complete per core
so mx/S are core-local; no collectives.  Each core's T matrix is rolled by
-p_slice so the diagonal band sits at fixed local columns [pb*128,(pb+1)*128)
(one SPMD program for all cores).

Per block (fp8_pass1=True, the default): pass 1 computes the row max from an
fp8e4m3 DoubleRow matmul (K=256 in ONE matmul at 0.5 cyc/row — 4x less PE
time than fp16, and mx only sets the softmax temperature so fp8 error is
acceptable: measured 6.6e-05 final rel err vs 3.6e-07 for the all-fp16
path); PSUM tiles are freed by the DVE max alone, so no staging copies are
needed at all.  Pass 2 recomputes dot in fp16 and ACT exps it straight from
PSUM with per-partition scale/bias (scale folds 5/denom, bias folds
10-5/denom AND the affine (1-dot)/2) and accum_out row-sums; the diagonal
band comes from an identity-mask multiply (GPSIMD) + row reduce (DVE).
Pass 2 lags pass 1 by one block so the mx->s/b chain is off the critical
path; the [128,1] chain ops run on the otherwise-idle GPSIMD.
A staged-fp16 fallback path (fp8_pass1=False: ACT copies H0 columns of each
block's PSUM to SBUF, PE recomputes the rest) is kept for reference.
Outputs per core: [128, 16] = (diag per block || S per block); host combines.

Pass-1 and pass-2 work is emitted interleaved at TILE granularity
(block_step_fp8), so ACT's exps trail the matmul stream by ~1 tile and the
kernel tail is short; the mx-combine runs on GPSIMD via tensor_scalar-max
(latency-critical — on DVE it queues behind the big PSUM reduces).

Engine budget per core (cost model, fp8 path): DVE ~40us (72%, row-max
from PSUM at 1 elem/lane/cycle is the floor), ACT ~39us, PE ~35us,
~55.4us wall.  The walrus build here allows only ONE sync
wait per instruction; _legalize_waits hoists extras onto EventSemaphore
carriers.
"""

import numpy as np
import ml_dtypes

# ---- problem constants (hardcoded; grading env has only this file) ----
N_SAMP, C, H, W = 2, 256, 64, 64
P = H * W                      # 4096
N_CORES = 8
CORES_PER_SAMPLE = 4
PSL = P // CORES_PER_SAMPLE    # 1024 p rows per core
PBLK = 128
NBLK = PSL // PBLK             # 8 blocks per core
KCH = C // 128                 # 2 contraction chunks
H0 = 2048                      # q columns staged via ACT copy (2 psum tiles)
H1 = P - H0                    # q columns recomputed for the exp pass
QT = 1024                      # psum tile width (2 banks)
FD = 512                       # matmul free-dim chunk (1 bank)
EPS = 1e-5

_CACHE = {}


def _legalize_waits(nc, max_waits=1):
    """The pinned walrus rejects instructions with more than one sync wait
    ("Too many sync wait commands").  Hoist excess waits onto standalone
    EventSemaphore carrier instructions on the same engine queue (queue is
    in-order, so a preceding wait is equivalent)."""
    from concourse import mybir

    n = 0
    for fn in nc.m.functions:
        for blk in fn.blocks:
            new_insts = []
            for inst in blk.instructions:
                si = getattr(inst, "sync_info", None)
                waits = list(si.on_wait) if (si is not None and si.on_wait) else []
                if len(waits) > max_waits:
                    excess = waits[:len(waits) - max_waits]
                    si.on_wait = waits[len(waits) - max_waits:]
                    for w in excess:
                        n += 1
                        new_insts.append(mybir.InstEventSemaphore(
                            name=f"{inst.name}-lw{n}",
                            engine=inst.engine,
                            ins=[], outs=[],
                            sync_info=mybir.SyncInfo(on_wait=[w], on_update=[]),
                        ))
                new_insts.append(inst)
            blk.instructions = new_insts
    return n


def _build_nc(H0=1024, QT=QT, ps_bufs=4, stg_bufs=2, w_bufs=3, lag=1,
              max_from_psum=True, last_full=False, rch_plan=(512, 512, 512, 512, 1024, 1024),
              repeats=1, h0_sched=(2048, 1024, 1024, 1024, 1024, 1024, 1024, 1024),
              dve_copy=0, fp8_pass1=True, use_gpsimd=True, QT2=1024, QT1=1024,
              fast_recip=False, rch_plan16=(512, 512, 1024, 1024, 1024),
              ps_split=(2, 2)):
    import concourse.bass as bass
    from concourse import mybir
    from concourse.tile import TileContext

    f16 = mybir.dt.float16
    f32 = mybir.dt.float32
    Alu = mybir.AluOpType
    Act = mybir.ActivationFunctionType
    X = mybir.AxisListType.X

    # per-block staged-column counts; optionally the last block is fully
    # staged (no pass-2 recompute) to shorten the kernel tail
    H0s = [H0] * NBLK
    if last_full:
        H0s[NBLK - 1] = P
    if h0_sched is not None:
        H0s = list(h0_sched)
        assert len(H0s) == NBLK

    nc = bass.Bass()
    f8 = mybir.dt.float8e4
    lhs = nc.dram_tensor("lhs", [KCH, 128, PSL], f16, kind="ExternalInput")
    rhs = nc.dram_tensor("rhs", [KCH, 128, P], f16, kind="ExternalInput")
    if fp8_pass1:
        lhs8 = nc.dram_tensor("lhs8", [KCH, 128, PSL], f8, kind="ExternalInput")
        rhs8 = nc.dram_tensor("rhs8", [KCH, 128, P], f8, kind="ExternalInput")
    ident = nc.dram_tensor("ident", [PBLK, PBLK], f32, kind="ExternalInput")
    out = nc.dram_tensor("out", [PBLK, 2 * NBLK], f32, kind="ExternalOutput")


    with TileContext(nc) as tc:
        with (
            tc.tile_pool(name="singles", bufs=1) as singles,
            tc.tile_pool(name="stage", bufs=stg_bufs) as stage,
            tc.tile_pool(name="wpool", bufs=w_bufs) as wpool,
            tc.tile_pool(name="stats", bufs=4 * NBLK) as stats,
            tc.tile_pool(name="ps", bufs=ps_bufs, space="PSUM") as ps,
        ):
            # ---- one-time loads ----
            # fp8 copies first: pass 1 (DoubleRow max pass) needs them earliest
            if fp8_pass1:
                lhs8_sb = singles.tile([128, KCH, PSL], f8)
                nc.sync.dma_start(
                    out=lhs8_sb[:, :, :],
                    in_=lhs8[:, :, :].rearrange("k p n -> p k n"))
                rhs8_sb = []
                q8 = 0
                for j, rch in enumerate(rch_plan):
                    rc = singles.tile([128, KCH, rch], f8, tag=f"rhs8{j}")
                    nc.sync.dma_start(
                        out=rc[:, :, :],
                        in_=rhs8[:, :, q8:q8 + rch].rearrange("k p n -> p k n"))
                    rhs8_sb.append((q8, q8 + rch, rc))
                    q8 += rch

                def rhs8_at(q0):
                    for qs, qe, rc in rhs8_sb:
                        if qs <= q0 < qe:
                            assert q0 + FD <= qe
                            return rc, q0 - qs
                    raise AssertionError(q0)

            plan16 = rch_plan16 if rch_plan16 is not None else rch_plan
            lhs_sb = singles.tile([128, KCH, PSL], f16)
            for kc in range(KCH):
                nc.sync.dma_start(out=lhs_sb[:, kc, :], in_=lhs[kc, :, :])
            # rhs in separate per-chunk tiles so block-0 matmuls only wait on
            # their own chunk's DMA (single big DMA costs ~7us of startup)
            rhs_sb = []   # list of (q_start, q_end, tile)
            q = 0
            for j, rch in enumerate(plan16):
                rc = singles.tile([128, KCH, rch], f16, tag=f"rhs{j}")
                nc.sync.dma_start(
                    out=rc[:, :, :],
                    in_=rhs[:, :, q:q + rch].rearrange("k p n -> p k n"),
                )
                rhs_sb.append((q, q + rch, rc))
                q += rch
            assert q == P

            def rhs_at(q0):
                for qs, qe, rc in rhs_sb:
                    if qs <= q0 < qe:
                        assert q0 + FD <= qe
                        return rc, q0 - qs
                raise AssertionError(q0)
            ident_sb = singles.tile([PBLK, PBLK], f32)
            nc.gpsimd.dma_start(out=ident_sb, in_=ident[:, :])
            out_sb = singles.tile([PBLK, 2 * NBLK], f32)

            # per-block state carried across the software pipeline
            blk = [dict() for _ in range(NBLK)]

            def matmul_tile(pt, pb, q0):
                # pt[:, 0:QT] = lhs[:, pb-block].T @ rhs[:, q0:q0+QT]
                for kc in range(KCH):
                    for f in range(QT // FD):
                        rc, off = rhs_at(q0 + f * FD)
                        nc.tensor.matmul(
                            out=pt[:, f * FD:(f + 1) * FD],
                            lhsT=lhs_sb[:, kc, pb * PBLK:(pb + 1) * PBLK],
                            rhs=rc[:, kc, off:off + FD],
                            start=(kc == 0),
                            stop=(kc == KCH - 1),
                        )

            def matmul_tile8(pt, pb, q0, width):
                # one DoubleRow matmul covers the full K=256 contraction
                for f in range(width // FD):
                    rc, off = rhs8_at(q0 + f * FD)
                    nc.tensor.matmul(
                        out=pt[:, f * FD:(f + 1) * FD],
                        lhsT=lhs8_sb[:, :, pb * PBLK:(pb + 1) * PBLK],
                        rhs=rc[:, :, off:off + FD],
                        start=True, stop=True,
                        perf_mode=mybir.MatmulPerfMode.DoubleRow,
                    )

            def finish_chain(st, mx):
                # denom = (1-mx)/2 + eps ; s = 5/denom ; b = 10 - s
                # [128,1] chain ops go to the otherwise-idle GPSIMD (latency
                # is hidden by the one-block pass2 lag); reciprocal is
                # DVE-only so it stays there
                eng = nc.gpsimd if use_gpsimd else nc.vector
                denom = stats.tile([128, 1], f32, tag="denom")
                eng.tensor_scalar(
                    out=denom, in0=mx, scalar1=-0.5, scalar2=0.5 + EPS,
                    op0=Alu.mult, op1=Alu.add,
                )
                rec = stats.tile([128, 1], f32, tag="rec")
                if fast_recip:
                    # single custom-DVE op — REJECTED: custom-DVE ops fail
                    # this walrus's codegen ("ISA wrong length")
                    nc.vector.reciprocal_approx_fast(out=rec, in_=denom)
                else:
                    nc.vector.reciprocal(out=rec, in_=denom)
                b_ap = stats.tile([128, 1], f32, tag="b_ap")
                eng.tensor_scalar(
                    out=b_ap, in0=rec, scalar1=-5.0, scalar2=10.0,
                    op0=Alu.mult, op1=Alu.add,
                )
                s_ap = stats.tile([128, 1], f32, tag="s_ap")
                eng.tensor_scalar(
                    out=s_ap, in0=b_ap, scalar1=-1.0, scalar2=10.0,
                    op0=Alu.mult, op1=Alu.add,
                )
                st["s"], st["b"] = s_ap, b_ap

            def pass1_fp8(pb):
                st = blk[pb]
                mxp = stats.tile([128, P // QT], f32, tag="mxp")
                for j in range(P // QT):
                    pt = ps.tile([128, QT], f32, tag="pst")
                    matmul_tile8(pt, pb, j * QT, QT)
                    nc.vector.reduce_max(out=mxp[:, j:j + 1], in_=pt, axis=X)
                mx = stats.tile([128, 1], f32, tag="mx")
                nc.vector.reduce_max(out=mx, in_=mxp[:, 0:P // QT], axis=X)
                finish_chain(st, mx)

            def pass2_fp8(pb):
                st = blk[pb]
                s_ap, b_ap = st["s"], st["b"]
                nq = P // QT
                sp = stats.tile([128, nq], f32, tag="sp")
                for j in range(nq):
                    pt = ps.tile([128, QT], f32, tag="pst")
                    matmul_tile(pt, pb, j * QT)
                    w1 = wpool.tile([128, QT], f32, tag="w1")
                    nc.scalar.activation(
                        out=w1, in_=pt, func=Act.Exp,
                        bias=b_ap, scale=s_ap, accum_out=sp[:, j:j + 1],
                    )
                    if j == 0:
                        band = wpool.tile([PBLK, PBLK], f32, tag="band")
                        (nc.gpsimd if use_gpsimd else nc.vector).tensor_tensor(
                            out=band,
                            in0=w1[:, pb * PBLK:(pb + 1) * PBLK],
                            in1=ident_sb, op=Alu.mult,
                        )
                        nc.vector.tensor_reduce(
                            out=out_sb[:, pb:pb + 1], in_=band, axis=X, op=Alu.add,
                        )
                nc.vector.tensor_reduce(
                    out=out_sb[:, NBLK + pb:NBLK + pb + 1], in_=sp[:, 0:nq],
                    axis=X, op=Alu.add,
                )

            def pass1(pb):
                st = blk[pb]
                h0 = H0s[pb]
                n_q0, n_q1 = h0 // QT, (P - h0) // QT
                stg = stage.tile([128, h0], f32, tag="stg")
                nmx = (n_q0 if max_from_psum else 1) + n_q1
                mxp = stats.tile([128, nmx], f32, tag="mxp")
                # staged columns: matmul -> copy to SBUF.  The last `dve_copy`
                # columns are copied by DVE instead of ACT to offload the
                # busier engine (3-way balance of the 2nd materialization).
                for j in range(n_q0):
                    pt = ps.tile([128, QT], f32, tag="pst")
                    matmul_tile(pt, pb, j * QT)
                    if max_from_psum:
                        nc.vector.reduce_max(out=mxp[:, j:j + 1], in_=pt, axis=X)
                    dv = dve_copy if j == n_q0 - 1 else 0
                    dv = min(dv, QT)
                    if QT - dv > 0:
                        nc.scalar.activation(
                            out=stg[:, j * QT:(j + 1) * QT - dv],
                            in_=pt[:, 0:QT - dv], func=Act.Copy,
                        )
                    if dv > 0:
                        nc.vector.tensor_copy(
                            out=stg[:, (j + 1) * QT - dv:(j + 1) * QT],
                            in_=pt[:, QT - dv:QT],
                        )
                # recomputed columns: matmul -> DVE max only
                joff = n_q0 if max_from_psum else 1
                for j in range(n_q1):
                    pt = ps.tile([128, QT], f32, tag="pst")
                    matmul_tile(pt, pb, h0 + j * QT)
                    nc.vector.reduce_max(out=mxp[:, joff + j:joff + j + 1], in_=pt, axis=X)
                if not max_from_psum:
                    nc.vector.reduce_max(out=mxp[:, 0:1], in_=stg, axis=X)
                mx = stats.tile([128, 1], f32, tag="mx")
                nc.vector.reduce_max(out=mx, in_=mxp[:, 0:nmx], axis=X)
                # denom = (1-mx)/2 + eps ; s = 5/denom ; b = 10 - s
                denom = stats.tile([128, 1], f32, tag="denom")
                nc.vector.tensor_scalar(
                    out=denom, in0=mx, scalar1=-0.5, scalar2=0.5 + EPS,
                    op0=Alu.mult, op1=Alu.add,
                )
                rec = stats.tile([128, 1], f32, tag="rec")
                if fast_recip:
                    # single custom-DVE op — REJECTED: custom-DVE ops fail
                    # this walrus's codegen ("ISA wrong length")
                    nc.vector.reciprocal_approx_fast(out=rec, in_=denom)
                else:
                    nc.vector.reciprocal(out=rec, in_=denom)
                b_ap = stats.tile([128, 1], f32, tag="b_ap")
                nc.vector.tensor_scalar(
                    out=b_ap, in0=rec, scalar1=-5.0, scalar2=10.0,
                    op0=Alu.mult, op1=Alu.add,
                )
                s_ap = stats.tile([128, 1], f32, tag="s_ap")
                nc.vector.tensor_scalar(
                    out=s_ap, in0=b_ap, scalar1=-1.0, scalar2=10.0,
                    op0=Alu.mult, op1=Alu.add,
                )
                st["stg"], st["s"], st["b"] = stg, s_ap, b_ap

            def pass2(pb):
                st = blk[pb]
                h0 = H0s[pb]
                n_q1 = (P - h0) // QT
                stg, s_ap, b_ap = st["stg"], st["s"], st["b"]
                sp = stats.tile([128, 1 + n_q1], f32, tag="sp")
                w0 = wpool.tile([128, h0], f32, tag="w0")
                nc.scalar.activation(
                    out=w0, in_=stg, func=Act.Exp,
                    bias=b_ap, scale=s_ap, accum_out=sp[:, 0:1],
                )
                for j in range(n_q1):
                    pt = ps.tile([128, QT], f32, tag="pst")
                    matmul_tile(pt, pb, h0 + j * QT)
                    w1 = wpool.tile([128, QT], f32, tag="w1")
                    nc.scalar.activation(
                        out=w1, in_=pt, func=Act.Exp,
                        bias=b_ap, scale=s_ap, accum_out=sp[:, 1 + j:2 + j],
                    )
                # diagonal band: w0[:, pb*128:(pb+1)*128] . ident, row-reduced
                band = wpool.tile([PBLK, PBLK], f32, tag="band")
                nc.vector.tensor_tensor(
                    out=band,
                    in0=w0[:, pb * PBLK:(pb + 1) * PBLK],
                    in1=ident_sb,
                    op=Alu.mult,
                )
                nc.vector.tensor_reduce(
                    out=out_sb[:, pb:pb + 1], in_=band, axis=X, op=Alu.add,
                )
                nc.vector.tensor_reduce(
                    out=out_sb[:, NBLK + pb:NBLK + pb + 1], in_=sp[:, 0:1 + n_q1],
                    axis=X, op=Alu.add,
                )

            # software pipeline: pass2 lags pass1 by `lag` blocks
            # (repeats>1 unrolls the whole loop for steady-state timing runs)
            def block_step_fp8(b):
                """pass1(b) and pass2(b-1) interleaved at tile granularity:
                ACT exps trail the matmul stream by ~1 tile instead of a
                whole block, shortening the kernel tail.  Pass-2 tiles are
                QT2 wide (fewer ACT ops amortize the per-op bubble); pass-1
                tiles stay QT wide for cheap PSUM turnover."""
                st1 = blk[b] if b < NBLK else None
                st2 = blk[b - 1] if b >= 1 else None
                nq = P // QT1
                nq2 = P // QT2
                r = max(1, nq // nq2)  # pass-1 tiles per pass-2 tile
                if st1 is not None:
                    st1["mxp"] = stats.tile([128, nq], f32, tag="mxp", name="mxp_t")
                if st2 is not None:
                    st2["sp"] = stats.tile([128, nq2], f32, tag="sp", name="sp_t")
                    s_ap, b_ap = st2["s"], st2["b"]
                for j2 in range(nq2):
                    if st1 is not None:
                        for j in range(j2 * r, min(nq, (j2 + 1) * r)):
                            pt = ps.tile([128, QT1], f32, tag="pst",
                                         bufs=ps_split[0], name="pt1")
                            matmul_tile8(pt, b, j * QT1, QT1)
                            nc.vector.reduce_max(
                                out=st1["mxp"][:, j:j + 1], in_=pt, axis=X)
                    if st2 is not None:
                        pt = ps.tile([128, QT2], f32, tag="pst2",
                                     bufs=ps_split[1], name="pt2")
                        for f in range(QT2 // QT):
                            matmul_tile(pt[:, f * QT:(f + 1) * QT],
                                        b - 1, j2 * QT2 + f * QT)
                        w1 = wpool.tile([128, QT2], f32, tag="w1")
                        nc.scalar.activation(
                            out=w1, in_=pt, func=Act.Exp,
                            bias=b_ap, scale=s_ap,
                            accum_out=st2["sp"][:, j2:j2 + 1])
                        if j2 == 0:
                            band = wpool.tile([PBLK, PBLK], f32, tag="band")
                            (nc.gpsimd if use_gpsimd else nc.vector).tensor_tensor(
                                out=band,
                                in0=w1[:, (b - 1) * PBLK:b * PBLK],
                                in1=ident_sb, op=Alu.mult)
                            nc.vector.tensor_reduce(
                                out=out_sb[:, b - 1:b], in_=band,
                                axis=X, op=Alu.add)
                if st1 is not None:
                    # combine the 4 per-tile maxes on GPSIMD so the chain is
                    # not queued behind DVE's big PSUM reduces (Pool rejects
                    # TensorTensor-max, but TensorScalarPtr takes per-partition
                    # AP scalars and any ALU op)
                    mxp = st1["mxp"]
                    mx = stats.tile([128, 1], f32, tag="mx")
                    if use_gpsimd and nq == 4:
                        mh = stats.tile([128, 2], f32, tag="mh")
                        nc.gpsimd.tensor_scalar(
                            out=mh[:, 0:1], in0=mxp[:, 0:1],
                            scalar1=mxp[:, 1:2], scalar2=None, op0=Alu.max)
                        nc.gpsimd.tensor_scalar(
                            out=mh[:, 1:2], in0=mxp[:, 2:3],
                            scalar1=mxp[:, 3:4], scalar2=None, op0=Alu.max)
                        nc.gpsimd.tensor_scalar(
                            out=mx, in0=mh[:, 0:1],
                            scalar1=mh[:, 1:2], scalar2=None, op0=Alu.max)
                    elif use_gpsimd and nq == 2:
                        nc.gpsimd.tensor_scalar(
                            out=mx, in0=mxp[:, 0:1],
                            scalar1=mxp[:, 1:2], scalar2=None, op0=Alu.max)
                    else:
                        nc.vector.reduce_max(out=mx, in_=mxp, axis=X)
                    finish_chain(st1, mx)
                if st2 is not None:
                    sp = st2["sp"]
                    if use_gpsimd and nq2 == 2:
                        nc.gpsimd.tensor_scalar(
                            out=out_sb[:, NBLK + b - 1:NBLK + b], in0=sp[:, 0:1],
                            scalar1=sp[:, 1:2], scalar2=None, op0=Alu.add)
                    elif use_gpsimd and nq2 == 4:
                        sh = stats.tile([128, 2], f32, tag="sh", name="sh_t")
                        nc.gpsimd.tensor_scalar(
                            out=sh[:, 0:1], in0=sp[:, 0:1],
                            scalar1=sp[:, 1:2], scalar2=None, op0=Alu.add)
                        nc.gpsimd.tensor_scalar(
                            out=sh[:, 1:2], in0=sp[:, 2:3],
                            scalar1=sp[:, 3:4], scalar2=None, op0=Alu.add)
                        nc.gpsimd.tensor_scalar(
                            out=out_sb[:, NBLK + b - 1:NBLK + b], in0=sh[:, 0:1],
                            scalar1=sh[:, 1:2], scalar2=None, op0=Alu.add)
                    else:
                        nc.vector.tensor_reduce(
                            out=out_sb[:, NBLK + b - 1:NBLK + b],
                            in_=sp[:, 0:nq2], axis=X, op=Alu.add)

            # fp8_blocks[b]: which pass-1/2 implementation block b uses.
            # Hybrid default: block 0 takes the staged-fp16 path so ACT has
            # copy work during pipeline fill while fp8 blocks stream behind.
            if fp8_pass1 == "hybrid":
                fp8_blocks = [b >= 1 for b in range(NBLK)]
            else:
                fp8_blocks = [bool(fp8_pass1)] * NBLK
            for _ in range(repeats):
                if fp8_pass1 is True and lag == 1:
                    for b in range(NBLK + 1):
                        block_step_fp8(b)
                else:
                    for b in range(NBLK + lag):
                        if b < NBLK:
                            (pass1_fp8 if fp8_blocks[b] else pass1)(b)
                        if b >= lag:
                            bb = b - lag
                            (pass2_fp8 if fp8_blocks[bb] else pass2)(bb)

            nc.sync.dma_start(out=out[:, :], in_=out_sb)

    _legalize_waits(nc)
    return nc


def _prep_inputs(I_features, T_features):
    """Host-side feature normalization (fp64) + per-core sharding (fp16)."""
    I = np.asarray(I_features, dtype=np.float64)
    T = np.asarray(T_features, dtype=np.float64)
    meanT = T.mean(axis=(0, 2, 3), keepdims=True)
    Ic = I - meanT
    Tc = T - meanT
    In = Ic / np.sqrt((Ic * Ic).sum(axis=1, keepdims=True))
    Tn = Tc / np.sqrt((Tc * Tc).sum(axis=1, keepdims=True))
    Iv = In.reshape(N_SAMP, C, P).astype(np.float16)
    Tv = Tn.reshape(N_SAMP, C, P).astype(np.float16)

    ident = np.eye(PBLK, dtype=np.float32)
    in_maps = []
    for c in range(N_CORES):
        n = c // CORES_PER_SAMPLE
        sl = (c % CORES_PER_SAMPLE) * PSL
        lhs = Iv[n][:, sl:sl + PSL].reshape(KCH, 128, PSL)
        rhs = np.roll(Tv[n], -sl, axis=1).reshape(KCH, 128, P)
        in_maps.append({
            "lhs": np.ascontiguousarray(lhs),
            "rhs": np.ascontiguousarray(rhs),
            "lhs8": np.ascontiguousarray(lhs.astype(ml_dtypes.float8_e4m3)),
            "rhs8": np.ascontiguousarray(rhs.astype(ml_dtypes.float8_e4m3)),
            "ident": ident,
        })
    return in_maps


def _combine(results):
    """Host-side reduction of per-core (diag, S) partials to the loss."""
    losses = []
    for n in range(N_SAMP):
        ratios = []
        for cs in range(CORES_PER_SAMPLE):
            r = results[n * CORES_PER_SAMPLE + cs]["out"].astype(np.float64)
            diag = r[:, 0:NBLK]     # [128, 8]: q = sl + pb*128 + i
            ssum = r[:, NBLK:]
            ratios.append(diag / ssum)
        m = 0.5 + 0.5 * np.mean(ratios)
        losses.append(-np.log(m))
    return np.float32(np.mean(losses))


def kernel(I_features, T_features, _trace=False):
    from concourse.bass_utils import run_bass_kernel_spmd

    if "nc" not in _CACHE:
        _CACHE["nc"] = _build_nc()
    nc = _CACHE["nc"]

    in_maps = _prep_inputs(I_features, T_features)
    res = run_bass_kernel_spmd(
        nc, in_maps, core_ids=list(range(N_CORES)), trace=_trace,
    )
    if _trace:
        _CACHE["last_result"] = res
    return _combine(res.results)



# revision 22
# speedup vs baseline: 1.0378x; 1.0378x over previous
"""CXLoss kernel for trn2 (8 NeuronCores) — v3 single-pass design.

Math
----
reference computes, per sample n:
  dot[q,p]   = <Tn[:,q], In[:,p]>          (C=256 contraction, P=4096)
  raw        = (1-dot)/2
  mn[p]      = min_q raw[q,p];  denom = mn + eps
  w[q,p]     = exp((1 - raw/denom)/sigma)
  cx_feat    = w / sum_q w
  loss       = mean_n -log(mean_q max_p (0.5*cx_feat + 0.5*cx_sp)[q,p])

cx_sp (spatial branch) is input-independent and collapses to the identity in
fp32 (see kernel_baseline.py docstring), so only diag(cx_feat) and the
column sums S are needed:
  loss = mean_n(-log(mean_q(0.5 + 0.5*w[q,q]/S[q])))

Fold raw/mn into dot:  w = exp(dot*s_p + b_p) with
  denom = (1-mx_p)/2 + eps   (mx_p = max_q dot[q,p])
  s_p = 5/denom,  b_p = 10 - s_p       (sigma=0.1, B=1)

Sharding: 8 cores = 2 samples x 4 slices of the p axis (1024 rows each).
Per core, 8 p-blocks of 128 rows x 4096 q columns; q complete per core so
mx/S are core-local (no collectives).  T rolled by -p_slice so the diagonal
band sits at fixed local columns.

v3 design (vs the two-pass fp16 baseline): ONE fp8 DoubleRow matmul pass.
Each [128,2048] PSUM half-block is consumed by a DVE tensor_mask_reduce
(full-width mask) that writes the fp16 staged copy to SBUF *and* row-max-
accumulates in the same 1.04ns/elem PSUM read — staging is free and the
fp16 recompute matmul disappears.  The two halves chain through accum_in,
so mx pops out of the second mask_reduce with no combine op.  A tunable
number of halves per block (nb) are instead staged by ACT (Act.Copy) and
max'd by DVE from the staged fp16 at 2x (mask_reduce supports DVE mode
2x_1p) — trading ACT time for DVE time to balance the two fixed-assignment
engines (exp is ACT-only, max is DVE-only; GPSIMD cannot access PSUM and
walrus rejects wide max ALU ops on it; DMA cannot read PSUM).  B-halves go
to EARLY blocks where ACT would otherwise idle during pipeline fill.  The
per-block exp is a single [128,4096] fp16-SBUF ACT op with accum_out
giving S directly; diag comes from one DVE scalar_tensor_tensor
(band*ident, accum=sum) lagged 2 blocks so it never waits on a fresh exp.
The mx->s/b chain runs on the otherwise-idle GPSIMD (reciprocal via
normalize_recip's denom write-back) so it never queues behind DVE's
2.3us mask_reduces — on DVE the chain's serial hops cost ~5us/block of
ACT start latency.

NOTE: tensor_mask_reduce / normalize_recip are extended-ISA instructions;
concourse.library_overlay.lower_extended_insts(nc) must run before compile
or walrus fails with "ISA wrong length" (this was misdiagnosed as
"custom ops unsupported" in the baseline session).

Accuracy: all-fp8 dots staged through fp16 measured 1.27e-4 final rel err
vs the reference (host emulation; tolerance 2e-2).  Subsampling S is NOT
safe (~10% of rows have nearly all softmax mass on one element).
"""

import numpy as np
import ml_dtypes

# ---- problem constants (hardcoded; grading env has only this file) ----
N_SAMP, C, H, W = 2, 256, 64, 64
P = H * W                      # 4096
N_CORES = 8
CORES_PER_SAMPLE = 4
PSL = P // CORES_PER_SAMPLE    # 1024 p rows per core
PBLK = 128
NBLK = PSL // PBLK             # 8 blocks per core
KCH = C // 128                 # 2 contraction chunks
QT = 1024                      # psum tile width (2 banks)
NQT = P // QT                  # 4 tiles per block
FD = 512                       # matmul free-dim chunk (1 bank)
EPS = 1e-5

_CACHE = {}


def _legalize_waits(nc, max_waits=1):
    """The pinned walrus rejects instructions with more than one sync wait
    ("Too many sync wait commands").  Hoist excess waits onto standalone
    EventSemaphore carrier instructions on the same engine queue (queue is
    in-order, so a preceding wait is equivalent)."""
    from concourse import mybir

    n = 0
    for fn in nc.m.functions:
        for blk in fn.blocks:
            new_insts = []
            for inst in blk.instructions:
                si = getattr(inst, "sync_info", None)
                waits = list(si.on_wait) if (si is not None and si.on_wait) else []
                if len(waits) > max_waits:
                    excess = waits[:len(waits) - max_waits]
                    si.on_wait = waits[len(waits) - max_waits:]
                    for w in excess:
                        n += 1
                        new_insts.append(mybir.InstEventSemaphore(
                            name=f"{inst.name}-lw{n}",
                            engine=inst.engine,
                            ins=[], outs=[],
                            sync_info=mybir.SyncInfo(on_wait=[w], on_update=[]),
                        ))
                new_insts.append(inst)
            blk.instructions = new_insts
    return n


def _build_nc(nb=(0, 0, 0, 0, 0, 0, 0, 0), ps_bufs=4, stg_bufs=3, w_bufs=3,
              chain_on="pool", pool_recip=False, diag_lag=2,
              rch_plan=((1024, "act"), (1024, "sp"), (1024, "pool"),
                        (1024, "sp")),
              last_chain_dve=True, qt=QT):
    """nb[b]: number of ACT-staged (B-type) halves in block b, taken from the
    LOW half indices.  chain_on: 'pool' = s/b chain on GPSIMD (off the busy
    DVE queue), 'dve' = chain on DVE.  pool_recip: reciprocal via GPSIMD
    normalize_recip denom write-back instead of nc.vector.reciprocal."""
    import concourse.bass as bass
    from concourse import mybir
    from concourse.tile import TileContext
    from concourse.library_overlay import lower_extended_insts
    from concourse.dve_ops import OPS as DVE_OPS

    TMR = {o.name: o for o in DVE_OPS}["TENSOR_MASK_REDUCE"]
    nqt = P // qt

    f16 = mybir.dt.float16
    f32 = mybir.dt.float32
    f8 = mybir.dt.float8e4
    Alu = mybir.AluOpType
    Act = mybir.ActivationFunctionType
    X = mybir.AxisListType.X
    NEG = -3.0e38

    nc = bass.Bass()
    lhs8 = nc.dram_tensor("lhs8", [KCH, 128, PSL], f8, kind="ExternalInput")
    rhs8 = nc.dram_tensor("rhs8", [KCH, 128, P], f8, kind="ExternalInput")
    iot2 = nc.dram_tensor("iot2", [PBLK, 2], f32, kind="ExternalInput")
    out = nc.dram_tensor("out", [PBLK, 2 * NBLK], f32, kind="ExternalOutput")

    with TileContext(nc) as tc:
        with (
            tc.tile_pool(name="singles", bufs=1) as singles,
            tc.tile_pool(name="stg", bufs=stg_bufs) as stgp,
            tc.tile_pool(name="wp", bufs=w_bufs) as wp,
            tc.tile_pool(name="gout", bufs=2) as goutp,
            tc.tile_pool(name="band", bufs=2) as bandp,
            tc.tile_pool(name="stats", bufs=4 * NBLK) as stats,
            tc.tile_pool(name="ps", bufs=ps_bufs, space="PSUM") as ps,
        ):
            # ---- one-time loads ----
            # A DMA holds its issuing engine's SEQ for the WHOLE transfer
            # (~1.5-3us each), so spread the loads across idle queues:
            # SP gets lhs (block-0 slice first so matmul 0 starts early),
            # DVE gets the first rhs chunk (its first compute op waits on
            # that matmul anyway), Pool gets the rest.
            lhs8_sb = singles.tile([128, KCH, PSL], f8)
            nc.sync.dma_start(
                out=lhs8_sb[:, :, 0:PBLK],
                in_=lhs8[:, :, 0:PBLK].rearrange("k p n -> p k n"))
            nc.sync.dma_start(
                out=lhs8_sb[:, :, PBLK:],
                in_=lhs8[:, :, PBLK:].rearrange("k p n -> p k n"))
            mend = singles.tile([128, 1], f32)
            nc.gpsimd.memset(mend, float(qt))
            rhs8_sb = []
            q8 = 0
            engs = {"act": nc.scalar, "pool": nc.gpsimd, "sp": nc.sync}
            for j, (rch, ename) in enumerate(rch_plan):
                rc = singles.tile([128, KCH, rch], f8, tag=f"rhs8{j}")
                engs[ename].dma_start(
                    out=rc[:, :, :],
                    in_=rhs8[:, :, q8:q8 + rch].rearrange("k p n -> p k n"))
                rhs8_sb.append((q8, q8 + rch, rc))
                q8 += rch
            assert q8 == P

            def rhs8_at(q0):
                for qs, qe, rc in rhs8_sb:
                    if qs <= q0 < qe:
                        assert q0 + FD <= qe
                        return rc, q0 - qs
                raise AssertionError(q0)

            iot2_sb = singles.tile([PBLK, 2], f32)
            nc.gpsimd.dma_start(out=iot2_sb, in_=iot2[:, :])
            out_sb = singles.tile([PBLK, 2 * NBLK], f32)

            blk = [dict() for _ in range(NBLK)]

            def matmul_half(pt, pb, q0):
                # fp8 DoubleRow: one matmul per FD covers the K=256 contraction
                for f in range(qt // FD):
                    rc, off = rhs8_at(q0 + f * FD)
                    nc.tensor.matmul(
                        out=pt[:, f * FD:(f + 1) * FD],
                        lhsT=lhs8_sb[:, :, pb * PBLK:(pb + 1) * PBLK],
                        rhs=rc[:, :, off:off + FD],
                        start=True, stop=True,
                        perf_mode=mybir.MatmulPerfMode.DoubleRow,
                    )

            def chain(st, on_dve=False):
                # denom = (1-mx)/2+eps ; s = 5/denom ; b = 10-s
                # Normally on GPSIMD so it never queues behind DVE's big
                # mask_reduces; the LAST block's chain goes on DVE where it
                # runs back-to-back right after the final max (shorter tail).
                mx = st["mx"]
                ceng = nc.vector if (on_dve or chain_on != "pool") else nc.gpsimd
                denom = stats.tile([128, 1], f32, tag="denom")
                ceng.tensor_scalar(
                    out=denom, in0=mx, scalar1=-0.5, scalar2=0.5 + EPS,
                    op0=Alu.mult, op1=Alu.add)
                rec = stats.tile([128, 1], f32, tag="rec")
                if pool_recip and ceng is nc.gpsimd:
                    # normalize_recip overwrites its denom operand with the
                    # reciprocal; dummy result tile is discarded
                    dummy = stats.tile([128, 1], f32, tag="dummy")
                    nc.gpsimd.tensor_copy(out=rec, in_=denom)
                    nc.gpsimd.normalize_recip(out_ap=dummy, in_ap=denom,
                                              denom_ap=rec)
                else:
                    nc.vector.reciprocal(out=rec, in_=denom)
                s_ap = stats.tile([128, 1], f32, tag="s_ap")
                ceng.tensor_scalar(
                    out=s_ap, in0=rec, scalar1=5.0, scalar2=None, op0=Alu.mult)
                b_ap = stats.tile([128, 1], f32, tag="b_ap")
                ceng.tensor_scalar(
                    out=b_ap, in0=s_ap, scalar1=-1.0, scalar2=10.0,
                    op0=Alu.mult, op1=Alu.add)
                st["s"], st["b"] = s_ap, b_ap

            def production(b):
                st = blk[b]
                stg = stgp.tile([128, P], f16, tag="stg")
                acc_prev = None
                nB = nb[b]
                # A-halves (DVE PSUM reads) first in the accum chain so the
                # B-half's staged-f16 max lands LAST in DVE's queue — its ACT
                # stage op queues behind exp(b-1), so DVE must not need it
                # early.  B-half staging is emitted (and matmul'd) first so
                # ACT can stage as soon as its queue frees.
                order = list(range(nB, nqt)) + list(range(nB))
                for j in range(nB):
                    pt = ps.tile([128, qt], f32, tag="pst", name=f"ptB{b}_{j}")
                    matmul_half(pt, b, j * qt)
                    nc.scalar.activation(
                        out=stg[:, j * qt:(j + 1) * qt], in_=pt, func=Act.Copy)
                for j in order:
                    sl = stg[:, j * qt:(j + 1) * qt]
                    acc = stats.tile([128, 1], f32, tag="acc", name=f"acc{b}_{j}")
                    a_in = NEG if acc_prev is None else acc_prev
                    if j < nB:
                        # B-type: DVE re-maxes the ACT-staged f16 copy
                        g = goutp.tile([128, qt], f16, tag="gout")
                        nc.vector._custom_dve(
                            op=TMR, out=g, in0=sl, in1=mend,
                            s0=0.0, s1=a_in, imm2=1.0, accum_out=acc)
                    else:
                        # A-type: one DVE PSUM read stages fp16 AND maxes.
                        # Emitted via _custom_dve: the pinned bass_rust
                        # codegen for InstTensorMaskReduce bakes a stale
                        # dve-table row into the instruction bytes and the
                        # DVE crashes; InstCustomDveAnt encodes correctly.
                        # (C0=mask_start, C1=accum_in, C2=scale; mask_end
                        # rides in1 per the C3 spill.)
                        pt = ps.tile([128, qt], f32, tag="pst", name=f"ptA{b}_{j}")
                        matmul_half(pt, b, j * qt)
                        nc.vector._custom_dve(
                            op=TMR, out=sl, in0=pt, in1=mend,
                            s0=0.0, s1=a_in, imm2=1.0, accum_out=acc)
                    acc_prev = acc
                st["mx"] = acc_prev
                chain(st, on_dve=(last_chain_dve and b == NBLK - 1))
                st["stg"] = stg

            def consume_exp(b):
                st = blk[b]
                w = wp.tile([128, P], f16, tag="w")
                nc.scalar.activation(
                    out=w, in_=st["stg"], func=Act.Exp,
                    bias=st["b"], scale=st["s"],
                    accum_out=out_sb[:, NBLK + b:NBLK + b + 1])
                st["w"] = w

            def consume_diag(b):
                # per-partition mask [i, i+1) selects w[i, band+i] == the
                # diagonal element; max-accum with -inf init extracts it
                st = blk[b]
                band = bandp.tile([PBLK, PBLK], f16, tag="band")
                nc.vector._custom_dve(
                    op=TMR, out=band,
                    in0=st["w"][:, b * PBLK:(b + 1) * PBLK],
                    in1=iot2_sb[:, 1:2], s0=iot2_sb[:, 0:1], s1=NEG,
                    imm2=1.0, accum_out=out_sb[:, b:b + 1])

            # software pipeline: exp lags production by 1 block, diag by
            # diag_lag blocks (so DVE's stt never waits on a fresh exp)
            for b in range(NBLK + diag_lag):
                if diag_lag <= b:
                    consume_diag(b - diag_lag)
                if b < NBLK:
                    production(b)
                if 1 <= b < NBLK + 1:
                    consume_exp(b - 1)

            nc.sync.dma_start(out=out[:, :], in_=out_sb)

    _legalize_waits(nc)
    lower_extended_insts(nc)
    return nc


def _prep_inputs(I_features, T_features):
    """Host-side feature normalization (fp64) + per-core fp8 sharding."""
    I = np.asarray(I_features, dtype=np.float64)
    T = np.asarray(T_features, dtype=np.float64)
    meanT = T.mean(axis=(0, 2, 3), keepdims=True)
    Ic = I - meanT
    Tc = T - meanT
    In = Ic / np.sqrt((Ic * Ic).sum(axis=1, keepdims=True))
    Tn = Tc / np.sqrt((Tc * Tc).sum(axis=1, keepdims=True))
    Iv = In.reshape(N_SAMP, C, P).astype(ml_dtypes.float8_e4m3)
    Tv = Tn.reshape(N_SAMP, C, P).astype(ml_dtypes.float8_e4m3)

    iot2 = np.stack([np.arange(PBLK, dtype=np.float32),
                     np.arange(1, PBLK + 1, dtype=np.float32)], axis=1)
    in_maps = []
    for c in range(N_CORES):
        n = c // CORES_PER_SAMPLE
        sl = (c % CORES_PER_SAMPLE) * PSL
        lhs = Iv[n][:, sl:sl + PSL].reshape(KCH, 128, PSL)
        rhs = np.roll(Tv[n], -sl, axis=1).reshape(KCH, 128, P)
        in_maps.append({
            "lhs8": np.ascontiguousarray(lhs),
            "rhs8": np.ascontiguousarray(rhs),
            "iot2": iot2,
        })
    return in_maps


def _combine(results):
    """Host-side reduction of per-core (diag, S) partials to the loss."""
    losses = []
    for n in range(N_SAMP):
        ratios = []
        for cs in range(CORES_PER_SAMPLE):
            r = results[n * CORES_PER_SAMPLE + cs]["out"].astype(np.float64)
            diag = r[:, 0:NBLK]     # [128, 8]: q = sl + pb*128 + i
            ssum = r[:, NBLK:]
            ratios.append(diag / ssum)
        m = 0.5 + 0.5 * np.mean(ratios)
        losses.append(-np.log(m))
    return np.float32(np.mean(losses))


def kernel(I_features, T_features, _trace=False):
    from concourse.bass_utils import run_bass_kernel_spmd

    if "nc" not in _CACHE:
        _CACHE["nc"] = _build_nc()
    nc = _CACHE["nc"]

    in_maps = _prep_inputs(I_features, T_features)
    res = run_bass_kernel_spmd(
        nc, in_maps, core_ids=list(range(N_CORES)), trace=_trace,
    )
    if _trace:
        _CACHE["last_result"] = res
    return _combine(res.results)
